# revision 22
# baseline (speedup 1.0000x reference)
"""Trainium2 Bass kernel for CoordsSelect (batched voxel-feature gather).

reference semantics:
  volume: [B=4, F=16, D=120, D, D] f32, coords: [B, 3*A=6144] f32,
  num_atoms: [B] int32
  vox = floor(coords_xyz) (clipped to [0,119]); flat = ix*D*D + iy*D + iz
  out[b, f, a] = volume[b, f].flat[flat[b, a]] * (a < num_atoms[b])

Sharding: 8 cores = 4 batches x 2 atom-halves. Core c handles batch c//2,
atoms 1024*(c%2) .. 1024*(c%2)+1024, ALL 16 features.

Key idea vs the per-feature-window baseline: the host re-lays the volume as
vol2[w, f, v] = volume[f, 64*w + v] in bf16 (w < 27000 rows of
16 feat x 64 vox x 2B = 2KB), so ONE dma_gather descriptor per atom fetches
the 64-voxel window for all 16 features at once. Descriptor count per core
drops 16x (16384 -> 1024), which removes the GPSIMD ucode bottleneck
(~8ns/descriptor). bf16 halves both gather traffic and DVE select cost; the
one-hot select keeps exactly one nonzero per (atom, feature), so the only
error is the f32->bf16 cast of the volume (~0.4% rel, tolerance is 2e-2).

Index/layout plumbing (dma_gather ucode semantics): index position i lives
at idxs[i % 16, i // 16] (replicated across the 8 16-partition groups), and
gather output row i lands at out[i % 128, i // 128, :]. We assign position
i the atom a(i) = 8*(16*((i//16)%8) + i%16) + (i//16)//8, which makes:
  - gather out[q, j] = atom 8q + j -> 8 consecutive atoms per partition, so
    the final DRAM write (out[a, f] layout, transposed on host) is one DMA
    with 256B-contiguous runs, and the within-window selector comes from the
    natural contiguous coords layout crd2[q, u] = coords(atom 8q + u).
  - idxs[p, c] = w1[p, 8*(c%8) + c//8], a pure free-dim 8x8 transpose of
    w1[p, m] = row_id(atom 128*(m//8) + 8*(p%16) + m%8), whose coords layout
    crd1 the host pre-arranges (96B-contiguous DRAM runs).

floor(x) for x >= 0 is computed as x - mod(x, 1), and the within-window
offset as mod(flat, 64); invalid atoms (a >= num_atoms) get +65 added to
their selector so the one-hot misses the [0,64) iota and yields exact 0.
"""

import numpy as np
import ml_dtypes

import concourse.bass as bass
import concourse.mybir as mybir
import concourse.tile as tile
from concourse import bacc, library_config
from concourse.bass_utils import run_bass_kernel_spmd

B, F, D = 4, 16, 120
A = 2048
AH = A // 2             # atoms per core
D3 = D * D * D          # 1_728_000
NROWS = D3 // 64        # 27_000 rows of [16 feat, 64 vox]
ROWLEN = F * 64         # 1024 bf16 elems = 2KB per row
N_CORES = 8
NCHUNK = 2              # gather calls per core
CH = AH // NCHUNK       # indices per gather call (512)

f32 = mybir.dt.float32
bf16 = mybir.dt.bfloat16
i16 = mybir.dt.int16
i32 = mybir.dt.int32
Alu = mybir.AluOpType
AxisX = mybir.AxisListType.X

# FC kept for test.py compatibility (features are NOT sharded anymore)
FC = F


def build_bass(debug_dumps=False):
    """Build + compile the per-core Bass program (identical on all cores)."""
    nc = bacc.Bacc(
        "TRN2",
        target_bir_lowering=False,
        debug=False,
        num_devices=N_CORES,
    )

    vol = nc.dram_tensor("vol", [NROWS * ROWLEN], bf16, kind="ExternalInput")
    # coords pre-arranged on host into the two SBUF layouts we need
    crd1 = nc.dram_tensor("crd1", [128, 192], f32, kind="ExternalInput")
    crd2 = nc.dram_tensor("crd2", [128, 24], f32, kind="ExternalInput")
    # host consts: [14400,120,1] voxel weights, local atom ids, iota ramp,
    # and num_atoms rebased to this core's atom window (as f32)
    a0 = nc.dram_tensor("a0", [128, 8], i32, kind="ExternalInput")
    ce = nc.dram_tensor("ce", [128, 512], i32, kind="ExternalInput")
    natm = nc.dram_tensor("natm", [128, 1], i32, kind="ExternalInput")
    out = nc.dram_tensor("out", [AH * F], f32, kind="ExternalOutput")

    with tile.TileContext(nc) as tc:
        with (
            tc.tile_pool(name="p", bufs=1) as pool,
            tc.tile_pool(name="gp", bufs=2) as gpool,
            tc.tile_pool(name="sp", bufs=2) as spool,
        ):
            # dma_gather lives in the 'mlp' Q7 ucode library; load it first
            # (overlaps with the input DMAs + DVE index chain below).
            nc.gpsimd.load_library(library_config.mlp)

            # dummy 16-idx gather: the FIRST gather call pays a one-time
            # ~2.4us MOVE/DRAIN ring-setup penalty on the engine; paying it
            # here (only dep: the library) keeps it off the critical path.
            zidx = pool.tile([128, 1], i16)
            nc.gpsimd.memset(zidx[:], 0)
            dscr = pool.tile([128, 1, ROWLEN], bf16)
            nc.gpsimd.dma_gather(
                out_ap=dscr[:],
                in_ap=bass.AP(vol, 0, [[ROWLEN, NROWS], [1, ROWLEN]]),
                idxs_ap=zidx[:],
                num_idxs=16,
                num_idxs_reg=16,
                elem_size=ROWLEN,
                single_packet=False,
            )

            # ---- input loads, split across the two DMA-capable queues so
            # the critical-path crd1/wt arrive first ----
            crd1_t = pool.tile([128, 192], f32)
            nc.sync.dma_start(crd1_t[:], crd1.ap())
            crd2_t = pool.tile([128, 24], f32)
            nc.scalar.dma_start(crd2_t[:], crd2.ap())
            ce_t = pool.tile([128, 512], i32)
            nc.scalar.dma_start(ce_t[:], ce.ap())
            a0_t = pool.tile([128, 8], i32)
            nc.sync.dma_start(a0_t[:], a0.ap())
            natm_t = pool.tile([128, 1], i32)
            nc.scalar.dma_start(natm_t[:], natm.ap())

            # ---- chain 1: flat voxel ids in the gather-index layout ----
            # Pinned to the head of the Vector stream (high_priority) so the
            # scheduler cannot interleave chain-2 ops before it: the gather's
            # wait on the Vector semaphore then covers ONLY these 11 ops.
            # floor(x) via int-cast roundtrip (robust to cast rounding mode):
            # i = int(x); c = float(i); fl = c - (c > x)
            with tc.high_priority():
                ti1 = pool.tile([128, 192], i32)
                nc.vector.tensor_copy(out=ti1[:], in_=crd1_t[:])
                cc1 = pool.tile([128, 192], f32)
                nc.vector.tensor_copy(out=cc1[:], in_=ti1[:])
                gt1 = pool.tile([128, 192], f32)
                nc.vector.tensor_tensor(
                    out=gt1[:], in0=cc1[:], in1=crd1_t[:], op=Alu.is_gt
                )
                fl1c = pool.tile([128, 192], f32)
                nc.vector.tensor_tensor(
                    out=fl1c[:], in0=cc1[:], in1=gt1[:], op=Alu.subtract
                )
                # flat = (fx*120 + fy)*120 + fz, Horner on stride-3 views
                # (no extra weight-tensor input on the critical path)
                fv = fl1c[:].rearrange("p (m d) -> p m d", d=3)
                fl1 = pool.tile([128, 64], f32)
                nc.vector.tensor_scalar(
                    fl1[:], fv[:, :, 0:1].rearrange("p m o -> p (m o)"),
                    float(D), None, op0=Alu.mult,
                )
                nc.vector.tensor_tensor(
                    out=fl1[:], in0=fl1[:],
                    in1=fv[:, :, 1:2].rearrange("p m o -> p (m o)"),
                    op=Alu.add,
                )
                nc.vector.tensor_scalar(
                    fl1[:], fl1[:], float(D), None, op0=Alu.mult
                )
                nc.vector.tensor_tensor(
                    out=fl1[:], in0=fl1[:],
                    in1=fv[:, :, 2:3].rearrange("p m o -> p (m o)"),
                    op=Alu.add,
                )
                # exact int (flat < 2^24), row id = flat >> 6 (i32, bitVec
                # ops cannot cast), then i16 cast fused with the free-dim
                # 8x8 transpose: idxs[p, cu*8+cs] = w1[p, cs*8+cu]
                fi1 = pool.tile([128, 64], i32)
                nc.vector.tensor_copy(out=fi1[:], in_=fl1[:])
                w1i = pool.tile([128, 64], i32)
                nc.vector.tensor_scalar(
                    w1i[:], fi1[:], 6, None, op0=Alu.logical_shift_right
                )
                idxs = pool.tile([128, 64], i16)
                nc.vector.tensor_copy(
                    out=idxs[:].rearrange("p (cu cs) -> p cu cs", cs=8),
                    in_=w1i[:].rearrange("p (cs cu) -> p cu cs", cu=8),
                )

            # ---- chain 2: within-window selector in the output layout ----
            ti2 = pool.tile([128, 24], i32)
            nc.vector.tensor_copy(out=ti2[:], in_=crd2_t[:])
            cc2 = pool.tile([128, 24], f32)
            nc.vector.tensor_copy(out=cc2[:], in_=ti2[:])
            gt2 = pool.tile([128, 24], f32)
            nc.vector.tensor_tensor(
                out=gt2[:], in0=cc2[:], in1=crd2_t[:], op=Alu.is_gt
            )
            fl2c = pool.tile([128, 24], f32)
            nc.vector.tensor_tensor(
                out=fl2c[:], in0=cc2[:], in1=gt2[:], op=Alu.subtract
            )
            fv2 = fl2c[:].rearrange("p (m d) -> p m d", d=3)
            fl2 = pool.tile([128, 8], f32)
            nc.vector.tensor_scalar(
                fl2[:], fv2[:, :, 0:1].rearrange("p m o -> p (m o)"),
                float(D), None, op0=Alu.mult,
            )
            nc.vector.tensor_tensor(
                out=fl2[:], in0=fl2[:],
                in1=fv2[:, :, 1:2].rearrange("p m o -> p (m o)"), op=Alu.add
            )
            nc.vector.tensor_scalar(
                fl2[:], fl2[:], float(D), None, op0=Alu.mult
            )
            nc.vector.tensor_tensor(
                out=fl2[:], in0=fl2[:],
                in1=fv2[:, :, 2:3].rearrange("p m o -> p (m o)"), op=Alu.add
            )
            # within-window offset = flat & 63, integer path
            fi2 = pool.tile([128, 8], i32)
            nc.vector.tensor_copy(out=fi2[:], in_=fl2[:])
            win = pool.tile([128, 8], i32)
            nc.vector.tensor_scalar(
                win[:], fi2[:], 63, None, op0=Alu.bitwise_and
            )
            # invalid atoms: push selector out of the [0, 64) iota range
            pen = pool.tile([128, 8], i32)
            nc.vector.tensor_tensor(
                out=pen[:], in0=a0_t[:],
                in1=natm_t[:].to_broadcast([128, 8]), op=Alu.is_ge,
            )
            nc.vector.tensor_scalar(pen[:], pen[:], 65, None, op0=Alu.mult)
            nc.vector.tensor_tensor(
                out=win[:], in0=win[:], in1=pen[:], op=Alu.add
            )
            # one-hot oh[p, u, e] = (e == win[p, u]), bf16 (exact 0/1)
            oh = pool.tile([128, 8, 64], bf16)
            nc.vector.tensor_tensor(
                out=oh[:],
                in0=ce_t[:].rearrange("p (u e) -> p u e", e=64),
                in1=win[:].rearrange("p (u e) -> p u e", e=1).to_broadcast(
                    [128, 8, 64]
                ),
                op=Alu.is_equal,
            )

            # ---- per-chunk: gather -> select -> write, issued in chunk
            # order so each select's DMA-completion wait target only covers
            # the gathers issued so far (16 per chunk, not all 32).
            JC = AH // 128 // NCHUNK  # output rows per chunk per partition
            for k in range(NCHUNK):
                g_k = gpool.tile([128, JC, ROWLEN], bf16, name="g")
                nc.gpsimd.dma_gather(
                    out_ap=g_k[:],
                    in_ap=bass.AP(vol, 0, [[ROWLEN, NROWS], [1, ROWLEN]]),
                    idxs_ap=idxs[:, k * (CH // 16) : (k + 1) * (CH // 16)],
                    num_idxs=CH,
                    num_idxs_reg=CH,
                    elem_size=ROWLEN,
                    single_packet=False,
                )
                # chunk 0 on Vector; chunk 1 on GpSimd, which is idle once
                # its gather ucode is done -> the two selects run in parallel
                ceng = nc.vector if k == 0 else nc.gpsimd
                sel = spool.tile([128, JC, F, 64], bf16, name="sel")
                ceng.tensor_tensor(
                    out=sel[:],
                    in0=g_k[:].rearrange("p j (f e) -> p j f e", e=64),
                    in1=oh[:, k * JC : (k + 1) * JC, :]
                    .rearrange("p j (f e) -> p j f e", f=1)
                    .to_broadcast([128, JC, F, 64]),
                    op=Alu.mult,
                )
                w = 32
                while w >= 2:
                    ceng.tensor_tensor(
                        out=sel[:, :, :, 0:w],
                        in0=sel[:, :, :, 0:w],
                        in1=sel[:, :, :, w : 2 * w],
                        op=Alu.add,
                    )
                    w //= 2
                res = spool.tile([128, JC, F], f32, name="res")
                ceng.tensor_tensor(
                    out=res[:],
                    in0=sel[:, :, :, 0:1].rearrange("p j f o -> p j (f o)"),
                    in1=sel[:, :, :, 1:2].rearrange("p j f o -> p j (f o)"),
                    op=Alu.add,
                )
                # out[a, f] with a = 8q + k*JC + j: 64 contiguous f32 per q
                eng = nc.sync if k % 2 == 0 else nc.scalar
                eng.dma_start(
                    bass.AP(out, k * JC * F, [[8 * F, 128], [1, JC * F]]),
                    res[:],
                )

            if debug_dumps:
                d_idxs = nc.dram_tensor(
                    "d_idxs", [128, 64], i16, kind="ExternalOutput"
                )
                nc.sync.dma_start(d_idxs.ap(), idxs[:])
                d_win = nc.dram_tensor(
                    "d_win", [128, 8], f32, kind="ExternalOutput"
                )
                nc.sync.dma_start(d_win.ap(), win[:])
                d_fl1 = nc.dram_tensor(
                    "d_fl1", [128, 64], f32, kind="ExternalOutput"
                )
                nc.sync.dma_start(d_fl1.ap(), fl1[:])

    nc.compile()
    return nc


_NC_CACHE = None


def _get_nc():
    global _NC_CACHE
    if _NC_CACHE is None:
        _NC_CACHE = build_bass()
    return _NC_CACHE


def _consts():
    p = np.arange(128)
    a0 = (8 * p[:, None] + np.arange(8)[None, :]).astype(np.int32)
    ce = np.tile(
        np.tile(np.arange(64, dtype=np.int32), 8)[None, :], (128, 1)
    )                                                   # [128, 512]
    return a0, ce


def make_in_maps(volume, coords, num_atoms):
    a0, ce = _consts()
    # vol2[w, f, v] = volume[b, f, 64w + v] in bf16, shared per batch
    vols = []
    for b in range(B):
        v = np.asarray(volume[b], dtype=np.float32).reshape(F, NROWS, 64)
        vols.append(
            np.ascontiguousarray(v.transpose(1, 0, 2))
            .astype(ml_dtypes.bfloat16)
            .reshape(-1)
        )
    in_maps = []
    for c in range(N_CORES):
        b, h = c // 2, c % 2
        ch = np.asarray(coords[b], dtype=np.float32).reshape(A, 3)[
            h * AH : (h + 1) * AH
        ]                                               # [1024, 3]
        # crd1[p, (s u d)] = coords(atom 128s + 8*(p%16) + u)
        crd1_16 = np.ascontiguousarray(
            ch.reshape(8, 16, 8, 3).transpose(1, 0, 2, 3)
        ).reshape(16, 192)
        crd1 = np.tile(crd1_16, (8, 1))                 # replicate groups
        # crd2[q, (u d)] = coords(atom 8q + u): natural contiguous layout
        crd2 = np.ascontiguousarray(ch).reshape(128, 24)
        natm = np.full(
            (128, 1), int(num_atoms[b]) - h * AH, dtype=np.int32
        )
        in_maps.append(
            {
                "vol": vols[b],
                "crd1": crd1,
                "crd2": crd2,
                "a0": a0,
                "ce": ce,
                "natm": natm,
            }
        )
    return in_maps


def kernel(volume, coords, num_atoms):
    volume = np.asarray(volume, dtype=np.float32)
    coords = np.asarray(coords, dtype=np.float32)
    num_atoms = np.asarray(num_atoms, dtype=np.int32)

    nc = _get_nc()
    in_maps = make_in_maps(volume, coords, num_atoms)
    r = run_bass_kernel_spmd(nc, in_maps, core_ids=list(range(N_CORES)))

    out = np.empty((B, F, A), dtype=np.float32)
    for c, res in enumerate(r.results):
        b, h = c // 2, c % 2
        out[b, :, h * AH : (h + 1) * AH] = (
            np.asarray(res["out"], dtype=np.float32).reshape(AH, F).T
        )
    return out


# revision 23
# speedup vs baseline: 1.2846x; 1.2846x over previous
"""Trainium2 Bass kernel for CoordsSelect (batched voxel-feature gather).

reference semantics:
  volume: [B=4, F=16, D=120, D, D] f32, coords: [B, 3*A=6144] f32,
  num_atoms: [B] int32
  vox = floor(coords_xyz) (clipped to [0,119]); flat = ix*D*D + iy*D + iz
  out[b, f, a] = volume[b, f].flat[flat[b, a]] * (a < num_atoms[b])

Sharding: 8 cores = 4 batches x 2 atom-halves. Core c handles batch c//2,
atoms 1024*(c%2) .. 1024*(c%2)+1024, ALL 16 features.

Key idea vs the per-feature-window baseline: the host re-lays the volume as
vol2[w, f, v] = volume[f, 64*w + v] in bf16 (w < 27000 rows of
16 feat x 64 vox x 2B = 2KB), so ONE dma_gather descriptor per atom fetches
the 64-voxel window for all 16 features at once. Descriptor count per core
drops 16x (16384 -> 1024), which removes the GPSIMD ucode bottleneck
(~8ns/descriptor). bf16 halves both gather traffic and DVE select cost; the
one-hot select keeps exactly one nonzero per (atom, feature), so the only
error is the f32->bf16 cast of the volume (~0.4% rel, tolerance is 2e-2).

Index/layout plumbing (dma_gather ucode semantics): index position i lives
at idxs[i % 16, i // 16] (replicated across the 8 16-partition groups), and
gather output row i lands at out[i % 128, i // 128, :]. We assign position
i the atom a(i) = 8*(16*((i//16)%8) + i%16) + (i//16)//8, which makes:
  - gather out[q, j] = atom 8q + j -> 8 consecutive atoms per partition, so
    the final DRAM write (out[a, f] layout, transposed on host) is one DMA
    with 256B-contiguous runs, and the within-window selector comes from the
    natural contiguous coords layout crd2[q, u] = coords(atom 8q + u).
  - idxs[p, c] = w1[p, 8*(c%8) + c//8], a pure free-dim 8x8 transpose of
    w1[p, m] = row_id(atom 128*(m//8) + 8*(p%16) + m%8), whose coords layout
    crd1 the host pre-arranges (96B-contiguous DRAM runs).

floor(x) for x >= 0 is computed as x - mod(x, 1), and the within-window
offset as mod(flat, 64); invalid atoms (a >= num_atoms) get +65 added to
their selector so the one-hot misses the [0,64) iota and yields exact 0.
"""

import numpy as np
import ml_dtypes

import concourse.bass as bass
import concourse.mybir as mybir
import concourse.tile as tile
from concourse import bacc, library_config
from concourse.bass_utils import run_bass_kernel_spmd

B, F, D = 4, 16, 120
A = 2048
AH = A // 2             # atoms per core
D3 = D * D * D          # 1_728_000
NROWS = D3 // 64        # 27_000 rows of [16 feat, 64 vox]
ROWLEN = F * 64         # 1024 bf16 elems = 2KB per row
N_CORES = 8
NCHUNK = 2              # gather calls per core
CH = AH // NCHUNK       # indices per gather call (512)

f32 = mybir.dt.float32
bf16 = mybir.dt.bfloat16
i16 = mybir.dt.int16
i32 = mybir.dt.int32
Alu = mybir.AluOpType
AxisX = mybir.AxisListType.X

# FC kept for test.py compatibility (features are NOT sharded anymore)
FC = F


def build_bass(debug_dumps=False):
    """Build + compile the per-core Bass program (identical on all cores)."""
    nc = bacc.Bacc(
        "TRN2",
        target_bir_lowering=False,
        debug=False,
        num_devices=N_CORES,
    )

    vol = nc.dram_tensor("vol", [NROWS * ROWLEN], bf16, kind="ExternalInput")
    # coords pre-arranged on host into the two SBUF layouts we need
    crd1 = nc.dram_tensor("crd1", [128, 192], f32, kind="ExternalInput")
    crd2 = nc.dram_tensor("crd2", [128, 24], f32, kind="ExternalInput")
    # host consts: [14400,120,1] voxel weights, local atom ids, iota ramp,
    # and num_atoms rebased to this core's atom window (as f32)
    a0 = nc.dram_tensor("a0", [128, 8], i32, kind="ExternalInput")
    ce = nc.dram_tensor("ce", [128, 512], i32, kind="ExternalInput")
    natm = nc.dram_tensor("natm", [128, 1], i32, kind="ExternalInput")
    out = nc.dram_tensor("out", [AH * F], f32, kind="ExternalOutput")

    with tile.TileContext(nc) as tc:
        with (
            tc.tile_pool(name="dp", bufs=1) as dpool,
            tc.tile_pool(name="p", bufs=1) as pool,
            tc.tile_pool(name="gp", bufs=2) as gpool,
            tc.tile_pool(name="sp", bufs=2) as spool,
        ):
            # dma_gather lives in the 'mlp' Q7 ucode library; load it first
            # (overlaps with the input DMAs + DVE index chain below).
            nc.gpsimd.load_library(library_config.mlp)

            # dummy 16-idx gather: the FIRST gather call pays a one-time
            # ~2.4us MOVE/DRAIN ring-setup penalty on the engine; paying it
            # here (only dep: the library) keeps it off the critical path.
            zidx = dpool.tile([128, 1], i16)
            nc.gpsimd.memset(zidx[:], 0)
            dscr = dpool.tile([128, 1, ROWLEN], bf16)
            nc.gpsimd.dma_gather(
                out_ap=dscr[:],
                in_ap=bass.AP(vol, 0, [[ROWLEN, NROWS], [1, ROWLEN]]),
                idxs_ap=zidx[:],
                num_idxs=16,
                num_idxs_reg=16,
                elem_size=ROWLEN,
                single_packet=False,
            )

            # ---- input loads, split across the two DMA-capable queues so
            # the critical-path crd1/wt arrive first ----
            crd1_t = pool.tile([128, 192], f32)
            nc.sync.dma_start(crd1_t[:], crd1.ap())
            crd2_t = pool.tile([128, 24], f32)
            nc.scalar.dma_start(crd2_t[:], crd2.ap())
            ce_t = pool.tile([128, 512], i32)
            nc.scalar.dma_start(ce_t[:], ce.ap())
            a0_t = pool.tile([128, 8], i32)
            nc.sync.dma_start(a0_t[:], a0.ap())
            natm_t = pool.tile([128, 1], i32)
            nc.scalar.dma_start(natm_t[:], natm.ap())

            # ---- chain 1: flat voxel ids in the gather-index layout ----
            # Pinned to the head of the Vector stream (high_priority) so the
            # scheduler cannot interleave chain-2 ops before it: the gather's
            # wait on the Vector semaphore then covers ONLY these 11 ops.
            # floor(x) via int-cast roundtrip (robust to cast rounding mode):
            # i = int(x); c = float(i); fl = c - (c > x)
            with tc.high_priority():
                ti1 = pool.tile([128, 192], i32)
                nc.vector.tensor_copy(out=ti1[:], in_=crd1_t[:])
                cc1 = pool.tile([128, 192], f32)
                nc.vector.tensor_copy(out=cc1[:], in_=ti1[:])
                gt1 = pool.tile([128, 192], f32)
                nc.vector.tensor_tensor(
                    out=gt1[:], in0=cc1[:], in1=crd1_t[:], op=Alu.is_gt
                )
                fl1c = pool.tile([128, 192], f32)
                nc.vector.tensor_tensor(
                    out=fl1c[:], in0=cc1[:], in1=gt1[:], op=Alu.subtract
                )
                # flat = (fx*120 + fy)*120 + fz, Horner on stride-3 views
                # (no extra weight-tensor input on the critical path)
                fv = fl1c[:].rearrange("p (m d) -> p m d", d=3)
                fl1 = pool.tile([128, 64], f32)
                nc.vector.tensor_scalar(
                    fl1[:], fv[:, :, 0:1].rearrange("p m o -> p (m o)"),
                    float(D), None, op0=Alu.mult,
                )
                nc.vector.tensor_tensor(
                    out=fl1[:], in0=fl1[:],
                    in1=fv[:, :, 1:2].rearrange("p m o -> p (m o)"),
                    op=Alu.add,
                )
                nc.vector.tensor_scalar(
                    fl1[:], fl1[:], float(D), None, op0=Alu.mult
                )
                nc.vector.tensor_tensor(
                    out=fl1[:], in0=fl1[:],
                    in1=fv[:, :, 2:3].rearrange("p m o -> p (m o)"),
                    op=Alu.add,
                )
                # exact int (flat < 2^24), row id = flat >> 6 (i32, bitVec
                # ops cannot cast), then i16 cast fused with the free-dim
                # 8x8 transpose: idxs[p, cu*8+cs] = w1[p, cs*8+cu]
                fi1 = pool.tile([128, 64], i32)
                nc.vector.tensor_copy(out=fi1[:], in_=fl1[:])
                w1i = pool.tile([128, 64], i32)
                nc.vector.tensor_scalar(
                    w1i[:], fi1[:], 6, None, op0=Alu.logical_shift_right
                )
                idxs = pool.tile([128, 64], i16)
                nc.vector.tensor_copy(
                    out=idxs[:].rearrange("p (cu cs) -> p cu cs", cs=8),
                    in_=w1i[:].rearrange("p (cs cu) -> p cu cs", cu=8),
                )

            # ---- chain 2: within-window selector in the output layout ----
            ti2 = pool.tile([128, 24], i32)
            nc.vector.tensor_copy(out=ti2[:], in_=crd2_t[:])
            cc2 = pool.tile([128, 24], f32)
            nc.vector.tensor_copy(out=cc2[:], in_=ti2[:])
            gt2 = pool.tile([128, 24], f32)
            nc.vector.tensor_tensor(
                out=gt2[:], in0=cc2[:], in1=crd2_t[:], op=Alu.is_gt
            )
            fl2c = pool.tile([128, 24], f32)
            nc.vector.tensor_tensor(
                out=fl2c[:], in0=cc2[:], in1=gt2[:], op=Alu.subtract
            )
            fv2 = fl2c[:].rearrange("p (m d) -> p m d", d=3)
            fl2 = pool.tile([128, 8], f32)
            nc.vector.tensor_scalar(
                fl2[:], fv2[:, :, 0:1].rearrange("p m o -> p (m o)"),
                float(D), None, op0=Alu.mult,
            )
            nc.vector.tensor_tensor(
                out=fl2[:], in0=fl2[:],
                in1=fv2[:, :, 1:2].rearrange("p m o -> p (m o)"), op=Alu.add
            )
            nc.vector.tensor_scalar(
                fl2[:], fl2[:], float(D), None, op0=Alu.mult
            )
            nc.vector.tensor_tensor(
                out=fl2[:], in0=fl2[:],
                in1=fv2[:, :, 2:3].rearrange("p m o -> p (m o)"), op=Alu.add
            )
            # within-window offset = flat & 63, integer path
            fi2 = pool.tile([128, 8], i32)
            nc.vector.tensor_copy(out=fi2[:], in_=fl2[:])
            win = pool.tile([128, 8], i32)
            nc.vector.tensor_scalar(
                win[:], fi2[:], 63, None, op0=Alu.bitwise_and
            )
            # invalid atoms: push selector out of the [0, 64) iota range
            pen = pool.tile([128, 8], i32)
            nc.vector.tensor_tensor(
                out=pen[:], in0=a0_t[:],
                in1=natm_t[:].to_broadcast([128, 8]), op=Alu.is_ge,
            )
            nc.vector.tensor_scalar(pen[:], pen[:], 65, None, op0=Alu.mult)
            nc.vector.tensor_tensor(
                out=win[:], in0=win[:], in1=pen[:], op=Alu.add
            )
            # one-hot oh[p, u, e] = (e == win[p, u]), bf16 (exact 0/1)
            oh = pool.tile([128, 8, 64], bf16)
            nc.vector.tensor_tensor(
                out=oh[:],
                in0=ce_t[:].rearrange("p (u e) -> p u e", e=64),
                in1=win[:].rearrange("p (u e) -> p u e", e=1).to_broadcast(
                    [128, 8, 64]
                ),
                op=Alu.is_equal,
            )

            # ---- per-chunk: gather -> select -> write, issued in chunk
            # order so each select's DMA-completion wait target only covers
            # the gathers issued so far (16 per chunk, not all 32).
            JC = AH // 128 // NCHUNK  # output rows per chunk per partition
            for k in range(NCHUNK):
                g_k = gpool.tile([128, JC, ROWLEN], bf16, name="g")
                nc.gpsimd.dma_gather(
                    out_ap=g_k[:],
                    in_ap=bass.AP(vol, 0, [[ROWLEN, NROWS], [1, ROWLEN]]),
                    idxs_ap=idxs[:, k * (CH // 16) : (k + 1) * (CH // 16)],
                    num_idxs=CH,
                    num_idxs_reg=CH,
                    elem_size=ROWLEN,
                    single_packet=False,
                )
                ceng = nc.vector
                sel = spool.tile([128, JC, F, 64], bf16, name="sel")
                ceng.tensor_tensor(
                    out=sel[:],
                    in0=g_k[:].rearrange("p j (f e) -> p j f e", e=64),
                    in1=oh[:, k * JC : (k + 1) * JC, :]
                    .rearrange("p j (f e) -> p j f e", f=1)
                    .to_broadcast([128, JC, F, 64]),
                    op=Alu.mult,
                )
                # bf16 add-tree to width 8 (2x DVE mode), then one small
                # X-reduce: cheaper than tensor_reduce on the full width
                # (no 2x mode) or a full tree (per-instr overhead)
                w = 32
                while w >= 8:
                    ceng.tensor_tensor(
                        out=sel[:, :, :, 0:w],
                        in0=sel[:, :, :, 0:w],
                        in1=sel[:, :, :, w : 2 * w],
                        op=Alu.add,
                    )
                    w //= 2
                res = spool.tile([128, JC, F], f32, name="res")
                ceng.tensor_reduce(
                    out=res[:], in_=sel[:, :, :, 0:8], axis=AxisX, op=Alu.add
                )
                # out[a, f] with a = 8q + k*JC + j: 64 contiguous f32 per q
                eng = nc.sync if k % 2 == 0 else nc.scalar
                eng.dma_start(
                    bass.AP(out, k * JC * F, [[8 * F, 128], [1, JC * F]]),
                    res[:],
                )

            if debug_dumps:
                d_idxs = nc.dram_tensor(
                    "d_idxs", [128, 64], i16, kind="ExternalOutput"
                )
                nc.sync.dma_start(d_idxs.ap(), idxs[:])
                d_win = nc.dram_tensor(
                    "d_win", [128, 8], f32, kind="ExternalOutput"
                )
                nc.sync.dma_start(d_win.ap(), win[:])
                d_fl1 = nc.dram_tensor(
                    "d_fl1", [128, 64], f32, kind="ExternalOutput"
                )
                nc.sync.dma_start(d_fl1.ap(), fl1[:])

    nc.compile()
    return nc


_NC_CACHE = None


def _get_nc():
    global _NC_CACHE
    if _NC_CACHE is None:
        _NC_CACHE = build_bass()
    return _NC_CACHE


def _consts():
    p = np.arange(128)
    a0 = (8 * p[:, None] + np.arange(8)[None, :]).astype(np.int32)
    ce = np.tile(
        np.tile(np.arange(64, dtype=np.int32), 8)[None, :], (128, 1)
    )                                                   # [128, 512]
    return a0, ce


def make_in_maps(volume, coords, num_atoms):
    a0, ce = _consts()
    # vol2[w, f, v] = volume[b, f, 64w + v] in bf16, shared per batch
    vols = []
    for b in range(B):
        v = np.asarray(volume[b], dtype=np.float32).reshape(F, NROWS, 64)
        vols.append(
            np.ascontiguousarray(v.transpose(1, 0, 2))
            .astype(ml_dtypes.bfloat16)
            .reshape(-1)
        )
    in_maps = []
    for c in range(N_CORES):
        b, h = c // 2, c % 2
        ch = np.asarray(coords[b], dtype=np.float32).reshape(A, 3)[
            h * AH : (h + 1) * AH
        ]                                               # [1024, 3]
        # crd1[p, (s u d)] = coords(atom 128s + 8*(p%16) + u)
        crd1_16 = np.ascontiguousarray(
            ch.reshape(8, 16, 8, 3).transpose(1, 0, 2, 3)
        ).reshape(16, 192)
        crd1 = np.tile(crd1_16, (8, 1))                 # replicate groups
        # crd2[q, (u d)] = coords(atom 8q + u): natural contiguous layout
        crd2 = np.ascontiguousarray(ch).reshape(128, 24)
        natm = np.full(
            (128, 1), int(num_atoms[b]) - h * AH, dtype=np.int32
        )
        in_maps.append(
            {
                "vol": vols[b],
                "crd1": crd1,
                "crd2": crd2,
                "a0": a0,
                "ce": ce,
                "natm": natm,
            }
        )
    return in_maps


def kernel(volume, coords, num_atoms):
    volume = np.asarray(volume, dtype=np.float32)
    coords = np.asarray(coords, dtype=np.float32)
    num_atoms = np.asarray(num_atoms, dtype=np.int32)

    nc = _get_nc()
    in_maps = make_in_maps(volume, coords, num_atoms)
    r = run_bass_kernel_spmd(nc, in_maps, core_ids=list(range(N_CORES)))

    out = np.empty((B, F, A), dtype=np.float32)
    for c, res in enumerate(r.results):
        b, h = c // 2, c % 2
        out[b, :, h * AH : (h + 1) * AH] = (
            np.asarray(res["out"], dtype=np.float32).reshape(AH, F).T
        )
    return out
